# revision 9
# baseline (speedup 1.0000x reference)
"""Multi-head attention (B=8, N=1024, C=1024, H=16) on 8 TRN2 NeuronCores.

Sharding: data-parallel over batch B=8 -> one batch element per core.
Each core computes, for its batch element:
    qkv = x @ qkv_w.T ; q,k,v split ; per-head softmax(q k^T / sqrt(hd)) v

Device-side layout strategy (all matmuls contract over the SBUF partition dim):
  - host passes xT = x[b].T (bf16)     [C, N]   (c on partitions)
  - host passes wT = qkv_w.T (bf16)    [C, 3C]  (c on partitions)
  - qT/kT computed transposed          [d, n]   (head-dim on partitions)
  - v computed in natural layout       [n, dv]  (tokens on partitions), augmented
    with a ones-column so the PV matmul also yields the softmax denominator
  - scores computed transposed S^T=[j,i] in 2-j-tile chunks; exp fused into the
    PSUM->SBUF copy on the scalar engine (bf16 out) at chunk granularity so the
    PV accumulation pipelines with the score matmuls; O^T = v_aug.T @ E^T gives
    [hd+1, i] with row 64 the softmax row-sum; reciprocal done at [128,4]
    layout via a DRAM bounce (keeps the DVE reciprocal on all lanes),
    broadcast back via a stride-0 DRAM read; DMA out O^T.
  - host transposes the returned outT back to [n, c].

All matmul inputs are bf16 (PSUM accumulation in fp32); measured end-to-end
relative error ~5e-3 vs the fp32 reference (gate is 2e-2).
"""

import sys

sys.path.insert(0, "/opt/trn_rl_repo")

import ml_dtypes
import numpy as np

import concourse.bacc as bacc
import concourse.mybir as mybir
import concourse.tile as tile
from concourse.bass_utils import run_bass_kernel_spmd

F32 = mybir.dt.float32
BF16 = mybir.dt.bfloat16
EXP = mybir.ActivationFunctionType.Exp

N = 1024  # tokens
C = 1024  # channels
H = 16    # heads
HD = 64   # head dim
NB = 2    # n blocks of 512
CT = 8    # c tiles of 128
SCALE = HD ** -0.5


def build_nc():
    nc = bacc.Bacc(None, target_bir_lowering=False)
    xT_ext = nc.declare_dram_parameter("xT", [C, N], BF16, isOutput=False)
    wT_ext = nc.declare_dram_parameter("wT", [C, 3 * C], BF16, isOutput=False)
    outT_ext = nc.declare_dram_parameter("outT", [C, N], F32, isOutput=True)

    xT3 = xT_ext.rearrange("(co p) n -> p co n", p=128)    # [128, 8, 1024]
    wT3 = wT_ext.rearrange("(co p) d -> p co d", p=128)    # [128, 8, 3072]

    with tile.TileContext(nc) as tc:
        with (
            tc.tile_pool(name="singles", bufs=1) as singles,
            tc.tile_pool(name="psum", bufs=1, space="PSUM") as psum,
            tc.tile_pool(name="drp", bufs=4, space="DRAM") as drp,
            tc.tile_pool(name="wqkpool", bufs=2) as wqkpool,
            tc.tile_pool(name="qkpool", bufs=2) as qkpool,
            tc.tile_pool(name="epool", bufs=4) as epool,
            tc.tile_pool(name="opool", bufs=3) as opool,
        ):
            # ---- load xT (kept resident), split per c-tile for early start ----
            xT_sb = singles.tile([128, CT, N], BF16)
            for ct in range(CT):
                nc.sync.dma_start(out=xT_sb[:, ct, :], in_=xT3[:, ct, :])

            # v_aug[p, nt, h, 0:64] = v head h rows; v_aug[p, nt, h, 64] = 1.0
            v_aug = singles.tile([128, CT, H, HD + 1], BF16)
            ones16 = singles.tile([128, H], F32)
            nc.vector.memset(ones16, 1.0)

            def load_wqk(t):
                wqk = wqkpool.tile([128, CT, 256], BF16, tag="wqk", name="wqk")
                nc.sync.dma_start(
                    out=wqk[:, :, 0:128], in_=wT3[:, :, t * 128:(t + 1) * 128]
                )
                nc.sync.dma_start(
                    out=wqk[:, :, 128:256],
                    in_=wT3[:, :, C + t * 128:C + (t + 1) * 128],
                )
                return wqk

            def qk_proj(wqk):
                # stationary (lhsT) = w tile, reused across the two n blocks
                qT = qkpool.tile([128, N], BF16, tag="qT", name="qT")
                kT = qkpool.tile([128, N], BF16, tag="kT", name="kT")
                for dst, off in ((qT, 0), (kT, 128)):
                    pss = [
                        psum.tile([128, 512], F32, tag="proj", bufs=2,
                                  name=f"qkps{nb}")
                        for nb in range(NB)
                    ]
                    for ct in range(CT):
                        for nb in range(NB):
                            nc.tensor.matmul(
                                pss[nb],
                                wqk[:, ct, off:off + 128],
                                xT_sb[:, ct, nb * 512:(nb + 1) * 512],
                                start=(ct == 0),
                                stop=(ct == CT - 1),
                            )
                    for nb in range(NB):
                        nc.vector.tensor_copy(
                            dst[:, nb * 512:(nb + 1) * 512], pss[nb]
                        )
                return qT, kT

            # ---- pair 0 qk projection first (so attention starts early) ----
            wqk0 = load_wqk(0)
            qkT = qk_proj(wqk0)

            # ---- v projection: v[n, dv] = sum_c x[n,c] wv[dv,c] ----
            # stationary (lhsT) = xT tile, reused across the two dv blocks
            with tc.tile_pool(name="wvpool", bufs=1) as wvpool:
                wv_sb = wvpool.tile([128, CT, C], BF16)
                for ct in range(CT):
                    nc.sync.dma_start(
                        out=wv_sb[:, ct, :], in_=wT3[:, ct, 2 * C:3 * C]
                    )
                for nt in range(8):
                    pss = [
                        psum.tile([128, 512], F32, tag="proj", bufs=2,
                                  name=f"vps{dvb}")
                        for dvb in range(2)
                    ]
                    for ct in range(CT):
                        for dvb in range(2):
                            nc.tensor.matmul(
                                pss[dvb],
                                xT_sb[:, ct, nt * 128:(nt + 1) * 128],
                                wv_sb[:, ct, dvb * 512:(dvb + 1) * 512],
                                start=(ct == 0),
                                stop=(ct == CT - 1),
                            )
                    for dvb in range(2):
                        h0 = dvb * 8
                        nc.vector.tensor_copy(
                            v_aug[:, nt, h0:h0 + 8, 0:HD],
                            pss[dvb].rearrange("p (h e) -> p h e", h=8),
                        )
                    nc.vector.tensor_copy(v_aug[:, nt, :, HD], ones16)

            # ---- per head-pair: scores, softmax, PV (pipelined per j-chunk) ----
            def attention_block(t, qT, kT, ib):
                """S^T chunks -> exp -> PV accumulation, software-pipelined."""
                ibs = slice(ib * 512, (ib + 1) * 512)
                pvs = [
                    psum.tile([HD + 1, 512], F32, tag="pv", bufs=2,
                              name=f"pv{hh}")
                    for hh in range(2)
                ]
                echunks = {}  # (hh, g) -> E tile [128, 2, 512]

                def s_chunk(g, hh):
                    p0 = hh * 64
                    sps = psum.tile([128, 1024], F32, tag="s", bufs=2,
                                    name=f"sps{hh}")
                    for jh in range(2):
                        jt = 2 * g + jh
                        nc.tensor.matmul(
                            sps[:, jh * 512:(jh + 1) * 512],
                            kT[p0:p0 + 64, jt * 128:(jt + 1) * 128],
                            qT[p0:p0 + 64, ibs],
                            start=True,
                            stop=True,
                        )
                    e = epool.tile([128, 2, 512], BF16, tag=f"E{hh}",
                                   name=f"E{hh}")
                    nc.scalar.activation(
                        out=e, in_=sps.rearrange("p (j f) -> p j f", j=2),
                        func=EXP, scale=SCALE,
                    )
                    echunks[(hh, g)] = e

                def pv_chunk(g, hh):
                    h = 2 * t + hh
                    e = echunks.pop((hh, g))
                    for jh in range(2):
                        jt = 2 * g + jh
                        nc.tensor.matmul(
                            pvs[hh],
                            v_aug[:, jt, h, :],
                            e[:, jh, :],
                            start=(g == 0 and jh == 0),
                            stop=(g == 3 and jh == 1),
                        )

                # software pipeline: S runs one chunk ahead of PV
                for hh in range(2):
                    s_chunk(0, hh)
                for g in range(1, 4):
                    for hh in range(2):
                        s_chunk(g, hh)
                    for hh in range(2):
                        pv_chunk(g - 1, hh)
                for hh in range(2):
                    pv_chunk(3, hh)

                for hh in range(2):
                    h = 2 * t + hh
                    pv = pvs[hh]
                    # normalize rows 0:64 by row 64 (the softmax sum):
                    # sums -> DRAM -> [128,4] so the reciprocal runs on all DVE
                    # lanes -> DRAM -> stride-0 partition broadcast -> multiply
                    srow = opool.tile([1, 512], F32, tag="srow", name="srow")
                    nc.vector.tensor_copy(srow, pv[HD:HD + 1, :])
                    dr = drp.tile([1, 512], F32, tag="dr", name="dr")
                    nc.sync.dma_start(out=dr, in_=srow)
                    r4 = opool.tile([128, 4], F32, tag="r4", name="r4")
                    nc.sync.dma_start(
                        out=r4, in_=dr.rearrange("o (p f) -> (o p) f", p=128)
                    )
                    r4b = opool.tile([128, 4], F32, tag="r4b", name="r4b")
                    nc.vector.reciprocal(r4b, r4)
                    dr2 = drp.tile([1, 512], F32, tag="dr2", name="dr2")
                    nc.sync.dma_start(
                        out=dr2.rearrange("o (p f) -> (o p) f", p=128), in_=r4b
                    )
                    bcast = opool.tile([HD, 512], F32, tag="bcast", name="bcast")
                    nc.sync.dma_start(
                        out=bcast, in_=dr2.partition_broadcast(HD).squeeze(1)
                    )
                    osb = opool.tile([HD, 512], F32, tag="osb", name="osb")
                    nc.vector.tensor_mul(osb, pv[0:HD, :], bcast)
                    nc.sync.dma_start(
                        out=outT_ext[h * HD:(h + 1) * HD, ibs], in_=osb
                    )

            for t in range(8):
                attention_block(t, qkT[0], qkT[1], 0)
                if t < 7:
                    wqk_next = load_wqk(t + 1)
                    qkT_next = qk_proj(wqk_next)
                else:
                    qkT_next = None
                attention_block(t, qkT[0], qkT[1], 1)
                qkT = qkT_next
    nc.compile()
    return nc


_NC_CACHE = {}


def _get_nc():
    if "nc" not in _NC_CACHE:
        _NC_CACHE["nc"] = build_nc()
    return _NC_CACHE["nc"]


def kernel(x: np.ndarray, qkv_w: np.ndarray, _trace: bool = False):
    B = x.shape[0]
    assert x.shape == (B, N, C) and qkv_w.shape == (3 * C, C)
    bf = ml_dtypes.bfloat16
    wT = np.ascontiguousarray(qkv_w.T).astype(bf)
    in_maps = [
        {"xT": np.ascontiguousarray(x[b].T).astype(bf), "wT": wT}
        for b in range(B)
    ]
    nc = _get_nc()
    res = run_bass_kernel_spmd(
        nc, in_maps, core_ids=list(range(8)), trace=_trace
    )
    out = np.stack([res.results[b]["outT"].T for b in range(B)])
    if _trace:
        return out, res
    return out


# revision 11
# speedup vs baseline: 1.2707x; 1.2707x over previous
"""Multi-head attention (B=8, N=1024, C=1024, H=16) on 8 TRN2 NeuronCores.

Sharding: data-parallel over batch B=8 -> one batch element per core.
Each core computes, for its batch element:
    qkv = x @ qkv_w.T ; q,k,v split ; per-head softmax(q k^T / sqrt(hd)) v

Device-side layout strategy (all matmuls contract over the SBUF partition dim):
  - host passes xT = x[b].T (bf16)     [C, N]   (c on partitions)
  - host passes wT = qkv_w.T (bf16)    [C, 3C]  (c on partitions)
  - qT/kT computed transposed          [d, n]   (head-dim on partitions)
  - v computed in natural layout       [n, dv]  (tokens on partitions), augmented
    with a ones-column so the PV matmul also yields the softmax denominator
  - scores computed transposed S^T=[j,i]; exp fused into the PSUM->SBUF copy
    on the scalar engine (bf16 out); O^T = v_aug.T @ E^T gives [hd+1, i] with
    row 64 the softmax row-sum; reciprocal done at [128,4] layout via a DRAM
    bounce, broadcast back via a stride-0 DRAM read.
  - host transposes the returned outT back to [n, c].

The attention phase is emitted as ONE globally software-pipelined stream:
score-chunks (2 matmuls + 1 exp), PV-chunks lagging a few chunks behind
(so the scalar-engine exp latency never stalls the PE), and the next pair's
qk-projection matmuls sprinkled two-per-chunk. PSUM: one shared 2-bank tag
(3 bufs) for scores+projection, and a 1-bank tag (2 bufs) for PV outputs,
which are released quickly by a staging copy before the normalization chain.

All matmul inputs are bf16 (PSUM accumulation in fp32); measured end-to-end
relative error ~5e-3 vs the fp32 reference (gate is 2e-2).
"""

import sys
from collections import deque

sys.path.insert(0, "/opt/trn_rl_repo")

import ml_dtypes
import numpy as np

import concourse.bacc as bacc
import concourse.mybir as mybir
import concourse.tile as tile
from concourse.bass_utils import run_bass_kernel_spmd

F32 = mybir.dt.float32
BF16 = mybir.dt.bfloat16
EXP = mybir.ActivationFunctionType.Exp

N = 1024  # tokens
C = 1024  # channels
H = 16    # heads
HD = 64   # head dim
NB = 2    # n blocks of 512
CT = 8    # c tiles of 128
SCALE = HD ** -0.5
PV_LAG = 3  # chunks the PV stream lags behind the score stream


def build_nc():
    nc = bacc.Bacc(None, target_bir_lowering=False)
    xT_ext = nc.declare_dram_parameter("xT", [C, N], BF16, isOutput=False)
    wT_ext = nc.declare_dram_parameter("wT", [C, 3 * C], BF16, isOutput=False)
    outT_ext = nc.declare_dram_parameter("outT", [C, N], F32, isOutput=True)

    xT3 = xT_ext.rearrange("(co p) n -> p co n", p=128)    # [128, 8, 1024]
    wT3 = wT_ext.rearrange("(co p) d -> p co d", p=128)    # [128, 8, 3072]

    with tile.TileContext(nc) as tc:
        with (
            tc.tile_pool(name="singles", bufs=1) as singles,
            tc.tile_pool(name="psum", bufs=1, space="PSUM") as psum,
            tc.tile_pool(name="drp", bufs=6, space="DRAM") as drp,
            tc.tile_pool(name="wqkpool", bufs=2) as wqkpool,
            tc.tile_pool(name="qkpool", bufs=2) as qkpool,
            tc.tile_pool(name="epool", bufs=6) as epool,
            tc.tile_pool(name="opool", bufs=4) as opool,
        ):
            # ---- load xT (kept resident), split per c-tile for early start ----
            xT_sb = singles.tile([128, CT, N], BF16)
            for ct in range(CT):
                nc.sync.dma_start(out=xT_sb[:, ct, :], in_=xT3[:, ct, :])

            # v_aug[p, nt, h, 0:64] = v head h rows; v_aug[p, nt, h, 64] = 1.0
            v_aug = singles.tile([128, CT, H, HD + 1], BF16)
            ones16 = singles.tile([128, H], F32)
            nc.vector.memset(ones16, 1.0)

            def ps_tile(name):
                return psum.tile([128, 1024], F32, tag="ps", bufs=3, name=name)

            def load_wqk(t):
                wqk = wqkpool.tile([128, CT, 256], BF16, tag="wqk", name="wqk")
                nc.sync.dma_start(
                    out=wqk[:, :, 0:128], in_=wT3[:, :, t * 128:(t + 1) * 128]
                )
                nc.sync.dma_start(
                    out=wqk[:, :, 128:256],
                    in_=wT3[:, :, C + t * 128:C + (t + 1) * 128],
                )
                return wqk

            def qk_proj_plain(wqk):
                """Non-pipelined qk projection (used for pair 0 only)."""
                qT = qkpool.tile([128, N], BF16, tag="qT", name="qT")
                kT = qkpool.tile([128, N], BF16, tag="kT", name="kT")
                for dst, off in ((qT, 0), (kT, 128)):
                    ps = ps_tile("qkps")
                    for ct in range(CT):
                        for nb in range(NB):
                            nc.tensor.matmul(
                                ps[:, nb * 512:(nb + 1) * 512],
                                wqk[:, ct, off:off + 128],
                                xT_sb[:, ct, nb * 512:(nb + 1) * 512],
                                start=(ct == 0),
                                stop=(ct == CT - 1),
                                skip_group_check=True,
                            )
                    nc.vector.tensor_copy(dst, ps)
                return qT, kT

            # ---- pair 0 qk projection first (so attention starts early) ----
            wqk0 = load_wqk(0)
            qkT = qk_proj_plain(wqk0)

            # ---- v projection: v[n, dv] = sum_c x[n,c] wv[dv,c] ----
            with tc.tile_pool(name="wvpool", bufs=1) as wvpool:
                wv_sb = wvpool.tile([128, CT, C], BF16)
                for ct in range(CT):
                    nc.sync.dma_start(
                        out=wv_sb[:, ct, :], in_=wT3[:, ct, 2 * C:3 * C]
                    )
                for nt in range(8):
                    ps = ps_tile("vps")
                    for ct in range(CT):
                        for dvb in range(2):
                            nc.tensor.matmul(
                                ps[:, dvb * 512:(dvb + 1) * 512],
                                xT_sb[:, ct, nt * 128:(nt + 1) * 128],
                                wv_sb[:, ct, dvb * 512:(dvb + 1) * 512],
                                start=(ct == 0),
                                stop=(ct == CT - 1),
                                skip_group_check=True,
                            )
                    for dvb in range(2):
                        h0 = dvb * 8
                        nc.vector.tensor_copy(
                            v_aug[:, nt, h0:h0 + 8, 0:HD],
                            ps[:, dvb * 512:(dvb + 1) * 512].rearrange(
                                "p (h e) -> p h e", h=8
                            ),
                        )
                    nc.vector.tensor_copy(v_aug[:, nt, :, HD], ones16)

            # ---- global software-pipelined attention stream ----
            pending = deque()  # FIFO of emitters: PV chunks and normalizations

            def drain(n_keep):
                while len(pending) > n_keep:
                    pending.popleft()()

            def normalize_emit(pv, h, ibs):
                def emit():
                    # release the pv bank quickly with a staging copy
                    stage = opool.tile([HD + 1, 512], F32, tag="stage",
                                       name="stage")
                    nc.vector.tensor_copy(stage, pv)
                    # sums -> DRAM -> [128,4] so the reciprocal runs on all
                    # DVE lanes -> DRAM -> stride-0 broadcast read -> multiply
                    dr = drp.tile([1, 512], F32, tag="dr", name="dr")
                    nc.sync.dma_start(out=dr, in_=stage[HD:HD + 1, :])
                    r4 = opool.tile([128, 4], F32, tag="r4", name="r4")
                    nc.sync.dma_start(
                        out=r4, in_=dr.rearrange("o (p f) -> (o p) f", p=128)
                    )
                    r4b = opool.tile([128, 4], F32, tag="r4b", name="r4b")
                    nc.vector.reciprocal(r4b, r4)
                    dr2 = drp.tile([1, 512], F32, tag="dr2", name="dr2")
                    nc.sync.dma_start(
                        out=dr2.rearrange("o (p f) -> (o p) f", p=128), in_=r4b
                    )
                    bcast = opool.tile([HD, 512], F32, tag="bcast", name="bcast")
                    nc.sync.dma_start(
                        out=bcast, in_=dr2.partition_broadcast(HD).squeeze(1)
                    )
                    osb = opool.tile([HD, 512], F32, tag="osb", name="osb")
                    nc.vector.tensor_mul(osb, stage[0:HD, :], bcast)
                    nc.sync.dma_start(
                        out=outT_ext[h * HD:(h + 1) * HD, ibs], in_=osb
                    )
                return emit

            for t in range(8):
                qT, kT = qkT
                # 2 projection matmuls per S-chunk: q during chunks 0..7,
                # k during chunks 8..15 of this pair's 16 chunks
                if t < 7:
                    wqk_next = load_wqk(t + 1)
                    qT_next = qkpool.tile([128, N], BF16, tag="qT", name="qT")
                    kT_next = qkpool.tile([128, N], BF16, tag="kT", name="kT")
                    proj_state = {}

                def proj_step(ci):
                    """Emit 2 projection matmuls for pair t+1 at chunk ci."""
                    if t >= 7:
                        return
                    half, step = divmod(ci, 8)  # 0: q, 1: k
                    dst, off = ((qT_next, 0), (kT_next, 128))[half]
                    if step == 0:
                        proj_state["ps"] = ps_tile("qkps")
                    ps = proj_state["ps"]
                    for nb in range(NB):
                        ct = step if half == 0 else step
                        nc.tensor.matmul(
                            ps[:, nb * 512:(nb + 1) * 512],
                            wqk_next[:, step, off:off + 128],
                            xT_sb[:, step, nb * 512:(nb + 1) * 512],
                            start=(step == 0),
                            stop=(step == CT - 1),
                            skip_group_check=True,
                        )
                    if step == CT - 1:
                        nc.vector.tensor_copy(dst, ps)

                pvs = {}
                ci = 0
                for ib in range(NB):
                    ibs = slice(ib * 512, (ib + 1) * 512)
                    for hh in range(2):
                        pvs[(ib, hh)] = psum.tile(
                            [HD + 1, 512], F32, tag="pv", bufs=2, name=f"pv{hh}"
                        )
                    for g in range(4):
                        for hh in range(2):
                            p0 = hh * 64
                            sps = ps_tile(f"sps{hh}")
                            for jh in range(2):
                                jt = 2 * g + jh
                                nc.tensor.matmul(
                                    sps[:, jh * 512:(jh + 1) * 512],
                                    kT[p0:p0 + 64, jt * 128:(jt + 1) * 128],
                                    qT[p0:p0 + 64, ibs],
                                    start=True,
                                    stop=True,
                                )
                            e = epool.tile([128, 2, 512], BF16, tag=f"E{hh}",
                                           name=f"E{hh}")
                            nc.scalar.activation(
                                out=e, in_=sps.rearrange("p (j f) -> p j f", j=2),
                                func=EXP, scale=SCALE,
                            )

                            def pv_emit(e=e, g=g, hh=hh, pv=pvs[(ib, hh)],
                                        h=2 * t + hh):
                                def emit():
                                    for jh in range(2):
                                        jt = 2 * g + jh
                                        nc.tensor.matmul(
                                            pv,
                                            v_aug[:, jt, h, :],
                                            e[:, jh, :],
                                            start=(g == 0 and jh == 0),
                                            stop=(g == 3 and jh == 1),
                                            skip_group_check=True,
                                        )
                                return emit

                            pending.append(pv_emit())
                            if g == 3:
                                pending.append(
                                    normalize_emit(pvs[(ib, hh)], 2 * t + hh, ibs)
                                )
                            proj_step(ci)
                            ci += 1
                            drain(PV_LAG)
                if t < 7:
                    qkT = (qT_next, kT_next)
            drain(0)
    nc.compile()
    return nc


_NC_CACHE = {}


def _get_nc():
    if "nc" not in _NC_CACHE:
        _NC_CACHE["nc"] = build_nc()
    return _NC_CACHE["nc"]


def kernel(x: np.ndarray, qkv_w: np.ndarray, _trace: bool = False):
    B = x.shape[0]
    assert x.shape == (B, N, C) and qkv_w.shape == (3 * C, C)
    bf = ml_dtypes.bfloat16
    wT = np.ascontiguousarray(qkv_w.T).astype(bf)
    in_maps = [
        {"xT": np.ascontiguousarray(x[b].T).astype(bf), "wT": wT}
        for b in range(B)
    ]
    nc = _get_nc()
    res = run_bass_kernel_spmd(
        nc, in_maps, core_ids=list(range(8)), trace=_trace
    )
    out = np.stack([res.results[b]["outT"].T for b in range(B)])
    if _trace:
        return out, res
    return out


# revision 12
# speedup vs baseline: 1.3039x; 1.0262x over previous
"""Multi-head attention (B=8, N=1024, C=1024, H=16) on 8 TRN2 NeuronCores.

Sharding: data-parallel over batch B=8 -> one batch element per core.
Each core computes, for its batch element:
    qkv = x @ qkv_w.T ; q,k,v split ; per-head softmax(q k^T / sqrt(hd)) v

Device-side layout strategy (all matmuls contract over the SBUF partition dim):
  - host passes xT = x[b].T (bf16)     [C, N]   (c on partitions)
  - host passes wT = qkv_w.T (bf16)    [C, 3C]  (c on partitions)
  - qT/kT computed transposed          [d, n]   (head-dim on partitions)
  - v computed in natural layout       [n, dv]  (tokens on partitions), augmented
    with a ones-column so the PV matmul also yields the softmax denominator
  - scores computed transposed S^T=[j,i]; exp fused into the PSUM->SBUF copy
    on the scalar engine (bf16 out); O^T = v_aug.T @ E^T gives [hd+1, i] with
    row 64 the softmax row-sum; reciprocal done at [128,4] layout via a DRAM
    bounce, broadcast back via a stride-0 DRAM read.
  - host transposes the returned outT back to [n, c].

The attention phase is emitted as ONE globally software-pipelined stream:
score-chunks (2 matmuls + 1 exp), PV-chunks lagging a few chunks behind
(so the scalar-engine exp latency never stalls the PE), and the next pair's
qk-projection matmuls sprinkled two-per-chunk. PSUM: one shared 2-bank tag
(3 bufs) for scores+projection, and a 1-bank tag (2 bufs) for PV outputs,
which are released quickly by a staging copy before the normalization chain.

All matmul inputs are bf16 (PSUM accumulation in fp32); measured end-to-end
relative error ~5e-3 vs the fp32 reference (gate is 2e-2).
"""

import sys
from collections import deque

sys.path.insert(0, "/opt/trn_rl_repo")

import ml_dtypes
import numpy as np

import concourse.bacc as bacc
import concourse.mybir as mybir
import concourse.tile as tile
from concourse.bass_utils import run_bass_kernel_spmd

F32 = mybir.dt.float32
BF16 = mybir.dt.bfloat16
EXP = mybir.ActivationFunctionType.Exp

N = 1024  # tokens
C = 1024  # channels
H = 16    # heads
HD = 64   # head dim
NB = 2    # n blocks of 512
CT = 8    # c tiles of 128
SCALE = HD ** -0.5
PV_LAG = 4  # chunks the PV stream lags behind the score stream


def build_nc():
    nc = bacc.Bacc(None, target_bir_lowering=False)
    xT_ext = nc.declare_dram_parameter("xT", [C, N], BF16, isOutput=False)
    # host-packed qk weights: [pair, p, co, 256] (q cols 0:128, k cols 128:256)
    wqk_ext = nc.declare_dram_parameter("wqk", [8, 128, CT, 256], BF16,
                                        isOutput=False)
    wv_ext = nc.declare_dram_parameter("wv", [C, C], BF16, isOutput=False)
    outT_ext = nc.declare_dram_parameter("outT", [C, N], F32, isOutput=True)

    xT3 = xT_ext.rearrange("(co p) n -> p co n", p=128)    # [128, 8, 1024]
    wv3 = wv_ext.rearrange("(co p) d -> p co d", p=128)    # [128, 8, 1024]

    with tile.TileContext(nc) as tc:
        with (
            tc.tile_pool(name="singles", bufs=1) as singles,
            tc.tile_pool(name="psum", bufs=1, space="PSUM") as psum,
            tc.tile_pool(name="drp", bufs=6, space="DRAM") as drp,
            tc.tile_pool(name="wqkpool", bufs=2) as wqkpool,
            tc.tile_pool(name="qkpool", bufs=2) as qkpool,
            tc.tile_pool(name="epool", bufs=6) as epool,
            tc.tile_pool(name="opool", bufs=4) as opool,
        ):
            # ---- prologue loads: pair-0 weights first, then xT per c-tile ----
            wqk_first = wqkpool.tile([128, CT, 256], BF16, tag="wqk",
                                     name="wqk_first")
            nc.sync.dma_start(out=wqk_first, in_=wqk_ext[0])
            xT_sb = singles.tile([128, CT, N], BF16)
            for ct in range(CT):
                nc.sync.dma_start(out=xT_sb[:, ct, :], in_=xT3[:, ct, :])

            # v_aug[p, nt, h, 0:64] = v head h rows; v_aug[p, nt, h, 64] = 1.0
            v_aug = singles.tile([128, CT, H, HD + 1], BF16)
            ones16 = singles.tile([128, H], F32)
            nc.vector.memset(ones16, 1.0)

            def ps_tile(name):
                return psum.tile([128, 1024], F32, tag="ps", bufs=3, name=name)

            def load_wqk(t):
                wqk = wqkpool.tile([128, CT, 256], BF16, tag="wqk", name="wqk")
                nc.sync.dma_start(out=wqk, in_=wqk_ext[t])
                return wqk

            def qk_proj_plain(wqk):
                """Non-pipelined qk projection (used for pair 0 only)."""
                qT = qkpool.tile([128, N], BF16, tag="qT", name="qT")
                kT = qkpool.tile([128, N], BF16, tag="kT", name="kT")
                for dst, off in ((qT, 0), (kT, 128)):
                    ps = ps_tile("qkps")
                    for ct in range(CT):
                        for nb in range(NB):
                            nc.tensor.matmul(
                                ps[:, nb * 512:(nb + 1) * 512],
                                wqk[:, ct, off:off + 128],
                                xT_sb[:, ct, nb * 512:(nb + 1) * 512],
                                start=(ct == 0),
                                stop=(ct == CT - 1),
                                skip_group_check=True,
                            )
                    nc.vector.tensor_copy(dst, ps)
                return qT, kT

            # ---- pair 0 qk projection first (so attention starts early) ----
            qkT = qk_proj_plain(wqk_first)

            # ---- v projection: v[n, dv] = sum_c x[n,c] wv[dv,c] ----
            with tc.tile_pool(name="wvpool", bufs=1) as wvpool:
                wv_sb = wvpool.tile([128, CT, C], BF16)
                for ct in range(CT):
                    nc.sync.dma_start(out=wv_sb[:, ct, :], in_=wv3[:, ct, :])
                for nt in range(8):
                    ps = ps_tile("vps")
                    for ct in range(CT):
                        for dvb in range(2):
                            nc.tensor.matmul(
                                ps[:, dvb * 512:(dvb + 1) * 512],
                                xT_sb[:, ct, nt * 128:(nt + 1) * 128],
                                wv_sb[:, ct, dvb * 512:(dvb + 1) * 512],
                                start=(ct == 0),
                                stop=(ct == CT - 1),
                                skip_group_check=True,
                            )
                    for dvb in range(2):
                        h0 = dvb * 8
                        nc.vector.tensor_copy(
                            v_aug[:, nt, h0:h0 + 8, 0:HD],
                            ps[:, dvb * 512:(dvb + 1) * 512].rearrange(
                                "p (h e) -> p h e", h=8
                            ),
                        )
                    nc.vector.tensor_copy(v_aug[:, nt, :, HD], ones16)

            # ---- global software-pipelined attention stream ----
            pending = deque()  # FIFO of emitters: PV chunks and normalizations

            def drain(n_keep):
                while len(pending) > n_keep:
                    pending.popleft()()

            def normalize_emit(pv, h, ibs):
                def emit():
                    # release the pv bank quickly with a staging copy
                    stage = opool.tile([HD + 1, 512], F32, tag="stage",
                                       name="stage")
                    nc.vector.tensor_copy(stage, pv)
                    # sums -> DRAM -> [128,4] so the reciprocal runs on all
                    # DVE lanes -> DRAM -> stride-0 broadcast read -> multiply
                    dr = drp.tile([1, 512], F32, tag="dr", name="dr")
                    nc.sync.dma_start(out=dr, in_=stage[HD:HD + 1, :])
                    r4 = opool.tile([128, 4], F32, tag="r4", name="r4")
                    nc.sync.dma_start(
                        out=r4, in_=dr.rearrange("o (p f) -> (o p) f", p=128)
                    )
                    r4b = opool.tile([128, 4], F32, tag="r4b", name="r4b")
                    nc.vector.reciprocal(r4b, r4)
                    dr2 = drp.tile([1, 512], F32, tag="dr2", name="dr2")
                    nc.sync.dma_start(
                        out=dr2.rearrange("o (p f) -> (o p) f", p=128), in_=r4b
                    )
                    bcast = opool.tile([HD, 512], F32, tag="bcast", name="bcast")
                    nc.sync.dma_start(
                        out=bcast, in_=dr2.partition_broadcast(HD).squeeze(1)
                    )
                    osb = opool.tile([HD, 512], F32, tag="osb", name="osb")
                    nc.vector.tensor_mul(osb, stage[0:HD, :], bcast)
                    nc.sync.dma_start(
                        out=outT_ext[h * HD:(h + 1) * HD, ibs], in_=osb
                    )
                return emit

            for t in range(8):
                qT, kT = qkT
                # 2 projection matmuls per S-chunk: q during chunks 0..7,
                # k during chunks 8..15 of this pair's 16 chunks
                if t < 7:
                    wqk_next = load_wqk(t + 1)
                    qT_next = qkpool.tile([128, N], BF16, tag="qT", name="qT")
                    kT_next = qkpool.tile([128, N], BF16, tag="kT", name="kT")
                    proj_state = {}

                def proj_step(ci):
                    """Emit 2 projection matmuls for pair t+1 at chunk ci."""
                    if t >= 7:
                        return
                    half, step = divmod(ci, 8)  # 0: q, 1: k
                    dst, off = ((qT_next, 0), (kT_next, 128))[half]
                    if step == 0:
                        proj_state["ps"] = ps_tile("qkps")
                    ps = proj_state["ps"]
                    for nb in range(NB):
                        ct = step if half == 0 else step
                        nc.tensor.matmul(
                            ps[:, nb * 512:(nb + 1) * 512],
                            wqk_next[:, step, off:off + 128],
                            xT_sb[:, step, nb * 512:(nb + 1) * 512],
                            start=(step == 0),
                            stop=(step == CT - 1),
                            skip_group_check=True,
                        )
                    if step == CT - 1:
                        nc.vector.tensor_copy(dst, ps)

                pvs = {}
                ci = 0
                for ib in range(NB):
                    ibs = slice(ib * 512, (ib + 1) * 512)
                    for hh in range(2):
                        pvs[(ib, hh)] = psum.tile(
                            [HD + 1, 512], F32, tag="pv", bufs=2, name=f"pv{hh}"
                        )
                    for g in range(4):
                        for hh in range(2):
                            p0 = hh * 64
                            sps = ps_tile(f"sps{hh}")
                            for jh in range(2):
                                jt = 2 * g + jh
                                nc.tensor.matmul(
                                    sps[:, jh * 512:(jh + 1) * 512],
                                    kT[p0:p0 + 64, jt * 128:(jt + 1) * 128],
                                    qT[p0:p0 + 64, ibs],
                                    start=True,
                                    stop=True,
                                )
                            e = epool.tile([128, 2, 512], BF16, tag=f"E{hh}",
                                           name=f"E{hh}")
                            nc.scalar.activation(
                                out=e.rearrange("p j f -> p (j f)"), in_=sps,
                                func=EXP, scale=SCALE,
                            )

                            def pv_emit(e=e, g=g, hh=hh, pv=pvs[(ib, hh)],
                                        h=2 * t + hh):
                                def emit():
                                    for jh in range(2):
                                        jt = 2 * g + jh
                                        nc.tensor.matmul(
                                            pv,
                                            v_aug[:, jt, h, :],
                                            e[:, jh, :],
                                            start=(g == 0 and jh == 0),
                                            stop=(g == 3 and jh == 1),
                                            skip_group_check=True,
                                        )
                                return emit

                            pending.append(pv_emit())
                            if g == 3:
                                pending.append(
                                    normalize_emit(pvs[(ib, hh)], 2 * t + hh, ibs)
                                )
                            proj_step(ci)
                            ci += 1
                            drain(PV_LAG if t < 7 else (2 if ib == 0 else 1))
                if t < 7:
                    qkT = (qT_next, kT_next)
            drain(0)
    nc.compile()
    return nc


_NC_CACHE = {}


def _get_nc():
    if "nc" not in _NC_CACHE:
        _NC_CACHE["nc"] = build_nc()
    return _NC_CACHE["nc"]


def kernel(x: np.ndarray, qkv_w: np.ndarray, _trace: bool = False):
    B = x.shape[0]
    assert x.shape == (B, N, C) and qkv_w.shape == (3 * C, C)
    bf = ml_dtypes.bfloat16
    # pack q,k weights: [pair, p, co, 256]; c = co*128 + p
    wq = qkv_w[0:C].T.reshape(CT, 128, 8, 128).transpose(2, 1, 0, 3)
    wk = qkv_w[C:2 * C].T.reshape(CT, 128, 8, 128).transpose(2, 1, 0, 3)
    wqk = np.ascontiguousarray(
        np.concatenate([wq, wk], axis=3)).astype(bf)
    wv = np.ascontiguousarray(qkv_w[2 * C:3 * C].T).astype(bf)
    in_maps = [
        {"xT": np.ascontiguousarray(x[b].T).astype(bf), "wqk": wqk, "wv": wv}
        for b in range(B)
    ]
    nc = _get_nc()
    res = run_bass_kernel_spmd(
        nc, in_maps, core_ids=list(range(8)), trace=_trace
    )
    out = np.stack([res.results[b]["outT"].T for b in range(B)])
    if _trace:
        return out, res
    return out


# revision 13
# speedup vs baseline: 1.5020x; 1.1520x over previous
"""Multi-head attention (B=8, N=1024, C=1024, H=16) on 8 TRN2 NeuronCores.

Sharding: data-parallel over batch B=8 -> one batch element per core.
Each core computes, for its batch element:
    qkv = x @ qkv_w.T ; q,k,v split ; per-head softmax(q k^T / sqrt(hd)) v

Device-side layout strategy (all matmuls contract over the SBUF partition dim):
  - host passes xT = x[b].T (bf16)     [C, N]   (c on partitions)
  - host passes wT = qkv_w.T (bf16)    [C, 3C]  (c on partitions)
  - qT/kT computed transposed          [d, n]   (head-dim on partitions)
  - v computed in natural layout       [n, dv]  (tokens on partitions), augmented
    with a ones-column so the PV matmul also yields the softmax denominator
  - scores computed transposed S^T=[j,i]; exp fused into the PSUM->SBUF copy
    on the scalar engine (bf16 out); O^T = v_aug.T @ E^T gives [hd+1, i] with
    row 64 the softmax row-sum; reciprocal done at [128,4] layout via a DRAM
    bounce, broadcast back via a stride-0 DRAM read.
  - host transposes the returned outT back to [n, c].

The attention phase is emitted as ONE globally software-pipelined stream:
score-chunks (2 matmuls + 1 exp), PV-chunks lagging a few chunks behind
(so the scalar-engine exp latency never stalls the PE), and the next pair's
qk-projection matmuls sprinkled two-per-chunk. PSUM: one shared 2-bank tag
(3 bufs) for scores+projection, and a 1-bank tag (2 bufs) for PV outputs,
which are released quickly by a staging copy before the normalization chain.

All matmul inputs are bf16 (PSUM accumulation in fp32); measured end-to-end
relative error ~5e-3 vs the fp32 reference (gate is 2e-2).
"""

import sys
from collections import deque

sys.path.insert(0, "/opt/trn_rl_repo")

import ml_dtypes
import numpy as np

import concourse.bacc as bacc
import concourse.mybir as mybir
import concourse.tile as tile
from concourse.bass_utils import run_bass_kernel_spmd

F32 = mybir.dt.float32
BF16 = mybir.dt.bfloat16
EXP = mybir.ActivationFunctionType.Exp

N = 1024  # tokens
C = 1024  # channels
H = 16    # heads
HD = 64   # head dim
NB = 2    # n blocks of 512
CT = 8    # c tiles of 128
SCALE = HD ** -0.5
PV_LAG = 6  # chunks the PV stream lags behind the score stream


def build_nc():
    nc = bacc.Bacc(None, target_bir_lowering=False)
    xT_ext = nc.declare_dram_parameter("xT", [C, N], BF16, isOutput=False)
    # host-packed qk weights: [pair, p, co, 256] (q cols 0:128, k cols 128:256)
    wqk_ext = nc.declare_dram_parameter("wqk", [8, 128, CT, 256], BF16,
                                        isOutput=False)
    wv_ext = nc.declare_dram_parameter("wv", [C, C], BF16, isOutput=False)
    outT_ext = nc.declare_dram_parameter("outT", [C, N], F32, isOutput=True)

    xT3 = xT_ext.rearrange("(co p) n -> p co n", p=128)    # [128, 8, 1024]
    wv3 = wv_ext.rearrange("(co p) d -> p co d", p=128)    # [128, 8, 1024]

    with tile.TileContext(nc) as tc:
        with (
            tc.tile_pool(name="singles", bufs=1) as singles,
            tc.tile_pool(name="psum", bufs=1, space="PSUM") as psum,
            tc.tile_pool(name="drp", bufs=6, space="DRAM") as drp,
            tc.tile_pool(name="wqkpool", bufs=2) as wqkpool,
            tc.tile_pool(name="qkpool", bufs=2) as qkpool,
            tc.tile_pool(name="epool", bufs=9) as epool,
            tc.tile_pool(name="opool", bufs=4) as opool,
        ):
            # ---- prologue loads: pair-0 weights first, then xT per c-tile ----
            wqk_first = wqkpool.tile([128, CT, 256], BF16, tag="wqk",
                                     name="wqk_first")
            nc.sync.dma_start(out=wqk_first, in_=wqk_ext[0])
            xT_sb = singles.tile([128, CT, N], BF16)
            for ct in range(CT):
                nc.sync.dma_start(out=xT_sb[:, ct, :], in_=xT3[:, ct, :])

            # v_aug[p, nt, h, 0:64] = v head h rows; v_aug[p, nt, h, 64] = 1.0
            v_aug = singles.tile([128, CT, H, HD + 1], BF16)
            ones16 = singles.tile([128, H], F32)
            nc.vector.memset(ones16, 1.0)

            def ps_tile(name):
                return psum.tile([128, 1024], F32, tag="ps", bufs=3, name=name)

            def load_wqk(t):
                wqk = wqkpool.tile([128, CT, 256], BF16, tag="wqk", name="wqk")
                nc.sync.dma_start(out=wqk, in_=wqk_ext[t])
                return wqk

            def qk_proj_plain(wqk):
                """Non-pipelined qk projection (used for pair 0 only)."""
                qT = qkpool.tile([128, N], BF16, tag="qT", name="qT")
                kT = qkpool.tile([128, N], BF16, tag="kT", name="kT")
                for dst, off in ((qT, 0), (kT, 128)):
                    ps = ps_tile("qkps")
                    for ct in range(CT):
                        for nb in range(NB):
                            nc.tensor.matmul(
                                ps[:, nb * 512:(nb + 1) * 512],
                                wqk[:, ct, off:off + 128],
                                xT_sb[:, ct, nb * 512:(nb + 1) * 512],
                                start=(ct == 0),
                                stop=(ct == CT - 1),
                                skip_group_check=True,
                            )
                    nc.vector.tensor_copy(dst, ps)
                return qT, kT

            # ---- pair 0 qk projection first (so attention starts early) ----
            qkT = qk_proj_plain(wqk_first)

            # ---- v projection: v[n, dv] = sum_c x[n,c] wv[dv,c] ----
            with tc.tile_pool(name="wvpool", bufs=1) as wvpool:
                wv_sb = wvpool.tile([128, CT, C], BF16)
                for ct in range(CT):
                    nc.sync.dma_start(out=wv_sb[:, ct, :], in_=wv3[:, ct, :])
                for nt in range(8):
                    ps = ps_tile("vps")
                    for ct in range(CT):
                        for dvb in range(2):
                            nc.tensor.matmul(
                                ps[:, dvb * 512:(dvb + 1) * 512],
                                xT_sb[:, ct, nt * 128:(nt + 1) * 128],
                                wv_sb[:, ct, dvb * 512:(dvb + 1) * 512],
                                start=(ct == 0),
                                stop=(ct == CT - 1),
                                skip_group_check=True,
                            )
                    for dvb in range(2):
                        h0 = dvb * 8
                        nc.vector.tensor_copy(
                            v_aug[:, nt, h0:h0 + 8, 0:HD],
                            ps[:, dvb * 512:(dvb + 1) * 512].rearrange(
                                "p (h e) -> p h e", h=8
                            ),
                        )
                    nc.vector.tensor_copy(v_aug[:, nt, :, HD], ones16)

            # ---- global software-pipelined attention stream ----
            pending = deque()  # FIFO of emitters: PV chunks and normalizations

            def drain(n_keep):
                while len(pending) > n_keep:
                    pending.popleft()()

            def normalize_emit(pv, h, ibs):
                def emit():
                    # release the pv bank quickly with a staging copy
                    stage = opool.tile([HD + 1, 512], F32, tag="stage",
                                       name="stage")
                    nc.vector.tensor_copy(stage, pv)
                    # sums -> DRAM -> [128,4] so the reciprocal runs on all
                    # DVE lanes -> DRAM -> stride-0 broadcast read -> multiply
                    dr = drp.tile([1, 512], F32, tag="dr", name="dr")
                    nc.sync.dma_start(out=dr, in_=stage[HD:HD + 1, :])
                    r4 = opool.tile([128, 4], F32, tag="r4", name="r4")
                    nc.sync.dma_start(
                        out=r4, in_=dr.rearrange("o (p f) -> (o p) f", p=128)
                    )
                    r4b = opool.tile([128, 4], F32, tag="r4b", name="r4b")
                    nc.vector.reciprocal(r4b, r4)
                    dr2 = drp.tile([1, 512], F32, tag="dr2", name="dr2")
                    nc.sync.dma_start(
                        out=dr2.rearrange("o (p f) -> (o p) f", p=128), in_=r4b
                    )
                    bcast = opool.tile([HD, 512], F32, tag="bcast", name="bcast")
                    nc.sync.dma_start(
                        out=bcast, in_=dr2.partition_broadcast(HD).squeeze(1)
                    )
                    osb = opool.tile([HD, 512], F32, tag="osb", name="osb")
                    nc.vector.tensor_mul(osb, stage[0:HD, :], bcast)
                    nc.sync.dma_start(
                        out=outT_ext[h * HD:(h + 1) * HD, ibs], in_=osb
                    )
                return emit

            for t in range(8):
                qT, kT = qkT
                # 2 projection matmuls per S-chunk: q during chunks 0..7,
                # k during chunks 8..15 of this pair's 16 chunks
                if t < 7:
                    wqk_next = load_wqk(t + 1)
                    qT_next = qkpool.tile([128, N], BF16, tag="qT", name="qT")
                    kT_next = qkpool.tile([128, N], BF16, tag="kT", name="kT")
                    proj_state = {}

                def proj_step(ci):
                    """4 projection matmuls for pair t+1, packed into chunks
                    0-3 (q) and 8-11 (k) so the psum slot is held only half
                    the pair."""
                    if t >= 7:
                        return
                    half, step4 = divmod(ci, 8)  # 0: q, 1: k
                    if step4 >= 4:
                        return
                    dst, off = ((qT_next, 0), (kT_next, 128))[half]
                    if step4 == 0:
                        proj_state["ps"] = ps_tile("qkps")
                    ps = proj_state["ps"]
                    for cth in range(2):
                        ct = step4 * 2 + cth
                        for nb in range(NB):
                            nc.tensor.matmul(
                                ps[:, nb * 512:(nb + 1) * 512],
                                wqk_next[:, ct, off:off + 128],
                                xT_sb[:, ct, nb * 512:(nb + 1) * 512],
                                start=(ct == 0),
                                stop=(ct == CT - 1),
                                skip_group_check=True,
                            )
                    if step4 == 3:
                        nc.vector.tensor_copy(dst, ps)

                pvs = {}
                ci = 0
                for ib in range(NB):
                    ibs = slice(ib * 512, (ib + 1) * 512)
                    for hh in range(2):
                        pvs[(ib, hh)] = psum.tile(
                            [HD + 1, 512], F32, tag="pv", bufs=2, name=f"pv{hh}"
                        )
                    for g in range(4):
                        for hh in range(2):
                            p0 = hh * 64
                            sps = ps_tile(f"sps{hh}")
                            for jh in range(2):
                                jt = 2 * g + jh
                                nc.tensor.matmul(
                                    sps[:, jh * 512:(jh + 1) * 512],
                                    kT[p0:p0 + 64, jt * 128:(jt + 1) * 128],
                                    qT[p0:p0 + 64, ibs],
                                    start=True,
                                    stop=True,
                                )
                            e = epool.tile([128, 2, 512], BF16, tag=f"E{hh}",
                                           name=f"E{hh}")
                            nc.scalar.activation(
                                out=e.rearrange("p j f -> p (j f)"), in_=sps,
                                func=EXP, scale=SCALE,
                            )

                            def pv_emit(e=e, g=g, hh=hh, pv=pvs[(ib, hh)],
                                        h=2 * t + hh):
                                def emit():
                                    for jh in range(2):
                                        jt = 2 * g + jh
                                        nc.tensor.matmul(
                                            pv,
                                            v_aug[:, jt, h, :],
                                            e[:, jh, :],
                                            start=(g == 0 and jh == 0),
                                            stop=(g == 3 and jh == 1),
                                            skip_group_check=True,
                                        )
                                return emit

                            pending.append(pv_emit())
                            if g == 3:
                                pending.append(
                                    normalize_emit(pvs[(ib, hh)], 2 * t + hh, ibs)
                                )
                            proj_step(ci)
                            ci += 1
                            drain(PV_LAG if t < 7 else (2 if ib == 0 else 1))
                if t < 7:
                    qkT = (qT_next, kT_next)
            drain(0)
    nc.compile()
    return nc


_NC_CACHE = {}


def _get_nc():
    if "nc" not in _NC_CACHE:
        _NC_CACHE["nc"] = build_nc()
    return _NC_CACHE["nc"]


def kernel(x: np.ndarray, qkv_w: np.ndarray, _trace: bool = False):
    B = x.shape[0]
    assert x.shape == (B, N, C) and qkv_w.shape == (3 * C, C)
    bf = ml_dtypes.bfloat16
    # pack q,k weights: [pair, p, co, 256]; c = co*128 + p
    wq = qkv_w[0:C].T.reshape(CT, 128, 8, 128).transpose(2, 1, 0, 3)
    wk = qkv_w[C:2 * C].T.reshape(CT, 128, 8, 128).transpose(2, 1, 0, 3)
    wqk = np.ascontiguousarray(
        np.concatenate([wq, wk], axis=3)).astype(bf)
    wv = np.ascontiguousarray(qkv_w[2 * C:3 * C].T).astype(bf)
    in_maps = [
        {"xT": np.ascontiguousarray(x[b].T).astype(bf), "wqk": wqk, "wv": wv}
        for b in range(B)
    ]
    nc = _get_nc()
    res = run_bass_kernel_spmd(
        nc, in_maps, core_ids=list(range(8)), trace=_trace
    )
    out = np.stack([res.results[b]["outT"].T for b in range(B)])
    if _trace:
        return out, res
    return out
